# revision 1
# baseline (speedup 1.0000x reference)
"""Trainium2 Bass kernel for nn_CrossAttention (b=2, sq=sk=2048, d=1024, h=16).

Sharding: 8 cores = 2 batches x 4 q-row blocks of 512. Each core computes all
16 heads for its q block plus the full output projection for those rows, so no
collectives are needed; host only slices/concats.

Per-core math (transposed-scores layout, so no on-device transposes):
  scoresT[s,q] = sum_d K[s,hd+d] * Q[q,hd+d]/8        (PE, fp16 in / fp32 psum)
  expT = exp(scoresT)                                  (ACT, psum->sbuf fp16)
  out2T[m,q] = sum_s v_aug[s,m] * expT[s,q]            (PE; v_aug = [V*km | km*64])
  outT[d,q] = out2T[d,q] * rs[q] / (denom[d%64,q]+eps) (DVE; denom rows 64..127)
  yT[j,q] = sum_d WoT[d,j] * outT[d,q] + bo[j]         (PE + DVE)

Key masking is folded into v_aug rows (multiplicative), so softmax needs no
-inf bias and exp can run over multi-bank PSUM spans. Query-mask / fully-masked
rows are zeroed by rs, matching the reference's nan_to_num semantics.
"""

import numpy as np

import concourse.mybir as mybir
import concourse.tile as tile
from concourse import bacc
from concourse import bass_utils

FP16 = mybir.dt.float16
F32 = mybir.dt.float32

# full-problem constants
B, SQ, SK, D, H, HD = 2, 2048, 2048, 1024, 16, 64
NCORES = 8
QBLK = SQ // 4  # 512 q rows per core


def _chunks(n_sk_tiles, parity):
    """Split sk tiles into PSUM-bank-sized chunks with an (size, tag) plan
    whose psum-slot reuse distance is always >=2, including across head
    boundaries: even heads run A,B,A,B,A and odd heads B,A,B,A,B."""
    if n_sk_tiles == 16:
        if parity == 0:
            return [(3, "A"), (4, "B"), (3, "A"), (4, "B"), (2, "A")]
        return [(4, "B"), (3, "A"), (4, "B"), (3, "A"), (2, "B")]
    # small-config fallback (sim tests)
    out = []
    rem = n_sk_tiles
    tag = "A" if parity == 0 else "B"
    while rem > 0:
        c = min(3 if tag == "A" else 4, rem)
        out.append((c, tag))
        rem -= c
        tag = "B" if tag == "A" else "A"
    return out


def build_program(h=H, qblk=QBLK, sk=SK, d=D, nq=None):
    """Build the per-core Bass program. Returns (nc, names)."""
    hd = HD
    skt = sk // 128  # sk tiles
    dch = d // 128  # d chunks (o-proj contraction / output chunks)
    nj = d // 128  # output feature chunks
    nc = bacc.Bacc(
        "TRN2",
        target_bir_lowering=False,
        debug=False,
        enable_asserts=False,
        num_devices=1,
    )

    qt = nc.dram_tensor("qt", [hd, h * qblk], FP16, kind="ExternalInput").ap()
    kt = nc.dram_tensor("kt", [h, hd, sk], FP16, kind="ExternalInput").ap()
    va = nc.dram_tensor("va", [h, 128, skt * 128], FP16, kind="ExternalInput").ap()
    wot = nc.dram_tensor("wot", [dch, 128, d], FP16, kind="ExternalInput").ap()
    bo = nc.dram_tensor("bo", [128, nj], F32, kind="ExternalInput").ap()
    rs = nc.dram_tensor("rs", [64, qblk], F32, kind="ExternalInput").ap()
    yt = nc.dram_tensor("yt", [nj, 128, qblk], F32, kind="ExternalOutput").ap()

    del nq

    with tile.TileContext(nc) as tc:
        with (
            tc.tile_pool(name="const", bufs=1) as cpool,
            tc.tile_pool(name="stream", bufs=3) as spool,
            tc.tile_pool(name="exp", bufs=4) as epool,
            tc.tile_pool(name="drain", bufs=2) as dpool,
            tc.tile_pool(name="p3", bufs=1, space="PSUM") as p3,
            tc.tile_pool(name="p4", bufs=1, space="PSUM") as p4,
            tc.tile_pool(name="pacc", bufs=1, space="PSUM") as pacc,
        ):
            qt_sb = cpool.tile([hd, h * qblk], FP16)
            outT_sb = cpool.tile([128, dch, qblk], FP16)
            wot_sb = cpool.tile([128, dch, d], FP16)
            bo_sb = cpool.tile([128, nj], F32)
            rs_sb = cpool.tile([64, qblk], F32)

            kt_sbs, va_sbs, av_pss = {}, {}, {}

            def load_head(hh):
                kt_sbs[hh] = spool.tile([hd, sk], FP16, tag="kt", name=f"kt_sb{hh}")
                nc.sync.dma_start(kt_sbs[hh][:], kt[hh])
                qsl = slice(hh * qblk, (hh + 1) * qblk)
                nc.sync.dma_start(qt_sb[:, qsl], qt[:, qsl])
                va_sbs[hh] = spool.tile([128, skt, 128], FP16, tag="va", name=f"va_sb{hh}")
                nc.sync.dma_start(
                    va_sbs[hh][:], va[hh].rearrange("p (t m) -> p t m", m=128)
                )

            def drain_head(hh):
                # evacuate PSUM fast (frees the accumulation bank for the
                # next head), then softmax-denominator math from SBUF
                av_sb = dpool.tile([128, qblk], F32, tag="avsb")
                nc.vector.tensor_copy(av_sb[:], av_pss[hh][:])
                sc = dpool.tile([64, qblk], F32, tag="sc")
                nc.vector.tensor_scalar_add(sc[:], av_sb[64:128, :], 1e-30)
                nc.vector.reciprocal(sc[:], sc[:])
                nc.vector.tensor_mul(sc[:], sc[:], rs_sb[:])
                chunk, half = hh // 2, (hh % 2) * 64
                nc.vector.tensor_tensor(
                    outT_sb[half : half + 64, chunk, :],
                    av_sb[0:64, :],
                    sc[:],
                    mybir.AluOpType.mult,
                )

            # flat, software-pipelined chunk stream: QK(c+1) is emitted
            # before AV(c) so the in-order PE queue never waits on exp(c)
            chunks = []
            for hh in range(h):
                t0 = 0
                for csz, tag in _chunks(skt, hh % 2):
                    chunks.append((hh, t0, csz, tag))
                    t0 += csz

            load_head(0)
            load_head(1)
            # constants needed only later; queue their DMAs after head data
            nc.sync.dma_start(wot_sb[:], wot.rearrange("c p j -> p c j"))
            nc.sync.dma_start(bo_sb[:], bo[:, :])
            nc.sync.dma_start(rs_sb[:], rs[:, :])

            def emit_av(item):
                ph, pt0, pcsz, pex = item
                for j in range(pcsz):
                    t = pt0 + j
                    nc.tensor.matmul(
                        av_pss[ph][:, :],
                        lhsT=va_sbs[ph][:, t, :],
                        rhs=pex[:, j * qblk : (j + 1) * qblk],
                        start=(t == 0),
                        stop=(t == skt - 1),
                    )
                if pt0 + pcsz == skt:
                    drain_head(ph)

            pending = []  # depth-2 queue of (hh, t0, csz, ex) awaiting AV
            for ci, (hh, t0, csz, tag) in enumerate(chunks):
                if t0 == 0:
                    if hh + 2 < h:
                        load_head(hh + 2)
                    av_pss[hh] = pacc.tile([128, qblk], F32, tag="acc", name=f"av_ps{hh}")
                pool = p3 if tag == "A" else p4
                qk_ps = pool.tile(
                    [128, csz * qblk], F32, tag="qk" + tag, name=f"qk_ps{ci}"
                )
                for j in range(csz):
                    t = t0 + j
                    nc.tensor.matmul(
                        qk_ps[:, j * qblk : (j + 1) * qblk],
                        lhsT=kt_sbs[hh][:, t * 128 : (t + 1) * 128],
                        rhs=qt_sb[:, hh * qblk : (hh + 1) * qblk],
                        start=True,
                        stop=True,
                    )
                if len(pending) == 2:
                    emit_av(pending.pop(0))
                ex = epool.tile([128, csz * qblk], FP16, tag="exp")
                nc.scalar.activation(ex[:], qk_ps[:], mybir.ActivationFunctionType.Exp)
                pending.append((hh, t0, csz, ex))

            for item in pending:
                emit_av(item)

            # output projection: yT[j,q] = sum_d WoT[d,j] outT[d,q] + bo[j]
            # (alternate accumulation between two pools -- the qk pools are
            # idle by now -- so consecutive j-chunks pipeline)
            for jc in range(nj):
                if jc % 2 == 0:
                    y_ps = pacc.tile([128, qblk], F32, tag="acc")
                else:
                    y_ps = p3.tile([128, qblk], F32, tag="qkA")
                for dc in range(dch):
                    nc.tensor.matmul(
                        y_ps[:],
                        lhsT=wot_sb[:, dc, jc * 128 : (jc + 1) * 128],
                        rhs=outT_sb[:, dc, :],
                        start=(dc == 0),
                        stop=(dc == dch - 1),
                    )
                y_sb = dpool.tile([128, qblk], F32, tag="y")
                nc.vector.tensor_tensor(
                    y_sb[:],
                    y_ps[:],
                    bo_sb[:, jc : jc + 1].to_broadcast((128, qblk)),
                    mybir.AluOpType.add,
                )
                nc.sync.dma_start(yt[jc], y_sb[:])

    nc.compile()
    return nc


def shard_inputs(query, key, value, key_mask, query_mask, Wo, bo):
    """Full inputs -> per-core input maps (host-side layout prep only)."""
    skt = SK // 128
    km01 = (key_mask[:, :, 0] != 0).astype(np.float32)  # [B, SK]
    any_km = km01.any(axis=1)  # [B]
    qm01 = (query_mask[:, :, 0] != 0).astype(np.float32)  # [B, SQ]

    woT = np.ascontiguousarray(Wo.astype(np.float32).T)  # [D, D] = [d, j]
    wot_t = woT.reshape(D // 128, 128, D).astype(np.float16)
    bo_r = np.ascontiguousarray(bo.astype(np.float32).reshape(D // 128, 128).T)

    in_maps = []
    for core in range(NCORES):
        g, r = core // 4, core % 4
        qs = slice(r * QBLK, (r + 1) * QBLK)

        q_blk = query[g, qs, :].astype(np.float32) * 0.125  # [QBLK, D]
        qt = np.ascontiguousarray(
            q_blk.reshape(QBLK, H, HD).transpose(2, 1, 0)  # [hd, h, q]
        ).reshape(HD, H * QBLK).astype(np.float16)

        kt = np.ascontiguousarray(
            key[g].astype(np.float32).reshape(SK, H, HD).transpose(1, 2, 0)
        ).astype(np.float16)  # [H, hd, SK]

        v_m = value[g].astype(np.float32) * km01[g][:, None]  # [SK, D]
        v_aug = np.empty((H, SK, 128), np.float32)
        v_aug[:, :, :64] = v_m.reshape(SK, H, HD).transpose(1, 0, 2)
        v_aug[:, :, 64:] = km01[g][None, :, None]
        va = np.ascontiguousarray(
            v_aug.reshape(H, skt, 128, 128).transpose(0, 2, 1, 3)  # [h, p, t, m]
        ).reshape(H, 128, skt * 128).astype(np.float16)

        rs = (qm01[g, qs] * (1.0 if any_km[g] else 0.0)).reshape(1, QBLK)
        rs = np.ascontiguousarray(np.broadcast_to(rs, (64, QBLK))).astype(np.float32)

        in_maps.append(
            {
                "qt": qt,
                "kt": kt,
                "va": va,
                "wot": wot_t,
                "bo": bo_r.astype(np.float32),
                "rs": rs,
            }
        )
    return in_maps


_NC_CACHE = {}


def _get_program():
    if "nc" not in _NC_CACHE:
        _NC_CACHE["nc"] = build_program()
    return _NC_CACHE["nc"]


def kernel(query, key, value, key_mask, query_mask, Wo, bo, _trace=False):
    query = np.asarray(query, dtype=np.float32)
    key = np.asarray(key, dtype=np.float32)
    value = np.asarray(value, dtype=np.float32)
    key_mask = np.asarray(key_mask, dtype=np.int32)
    query_mask = np.asarray(query_mask, dtype=np.int32)
    Wo = np.asarray(Wo, dtype=np.float32)
    bo = np.asarray(bo, dtype=np.float32)

    nc = _get_program()
    in_maps = shard_inputs(query, key, value, key_mask, query_mask, Wo, bo)
    try:
        res = bass_utils.run_bass_kernel_spmd(
            nc, in_maps, core_ids=list(range(NCORES)), trace=_trace
        )
    except ModuleNotFoundError:
        # axon NTFF profile hook unavailable in this container; run untraced
        res = bass_utils.run_bass_kernel_spmd(
            nc, in_maps, core_ids=list(range(NCORES)), trace=False
        )
    kernel.last_results = res

    out = np.empty((B, SQ, D), np.float32)
    for core in range(NCORES):
        g, r = core // 4, core % 4
        yt = res.results[core]["yt"]  # [nj, 128, QBLK]
        out[g, r * QBLK : (r + 1) * QBLK, :] = yt.reshape(D, QBLK).T
    return out



# revision 6
# speedup vs baseline: 2.9038x; 2.9038x over previous
"""Trainium2 Bass kernel for nn_CrossAttention (b=2, sq=sk=2048, d=1024, h=16).

The axon-tunneled host<->device link (~75 MB/s, high fixed cost) dominates
wall time, so the design minimizes transferred bytes: every unique input byte
is uploaded exactly once as fp16 (disjoint 1/8 slices packed into a single
blob per core), then redistributed on-device over NeuronLink via AllGather,
which is ~3 orders of magnitude faster than the host link.

Sharding: 8 cores = 2 batches x 4 q-row blocks of 512. Core (g, r) uploads
  - Q^T for its own q rows (pre-scaled by 1/8),
  - K rows  [r*512:(r+1)*512] of batch g  (transposed on-device, AllGathered
    within the 4-core batch group),
  - [V*km | km] rows of batch g           (AllGathered within batch group),
  - a 128-row slice of Wo^T               (AllGathered across all 8 cores),
  - rs/bo epilogue constants.
Each core then computes all 16 heads for its q block plus the output
projection; outputs are disjoint [512, 1024] fp16 blocks in natural layout.

Per-core math (transposed-scores layout):
  scoresT[s,q] = sum_d K[s,hd+d] * Q[q,hd+d]/8        (PE, fp16 in / fp32 psum)
  expT = exp(scoresT)                                  (ACT, psum->sbuf fp16)
  out2T[m,q] = sum_s v_aug[s,m] * expT[s,q]            (PE; v_aug = [V*km | km*64])
  outT[d,q] = out2T[d,q] * rs[q] / (denom[d%64,q]+eps) (DVE; denom rows 64..127)
  yT[j,q] = sum_d WoT[d,j] * outT[d,q] + bo[j]         (PE + DVE)
  y[q,j]  = yT[j,q]                                    (PE transpose, fp16 out)

Key masking is folded into v_aug rows (multiplicative) so softmax needs no
-inf bias; query-mask / fully-masked rows are zeroed by rs, matching the
reference's nan_to_num semantics.
"""

import numpy as np

import concourse.mybir as mybir
import concourse.tile as tile
from concourse import bacc
from concourse import bass_utils
from concourse.masks import make_identity

FP16 = mybir.dt.float16
F32 = mybir.dt.float32

# full-problem constants
B, SQ, SK, D, H, HD = 2, 2048, 2048, 1024, 16, 64
NCORES = 8
QBLK = SQ // 4  # 512 q rows per core
SLC = SK // 4  # 512 k/v rows per core

# blob layout (fp16 elements)
N_QT = HD * H * QBLK           # 524288   [64, 8192]   Q^T * 0.125
N_KN = SLC * D                 # 524288   [512, 1024]  K rows (natural)
N_VA = SLC * (D + HD)          # 557056   [512, 1088]  [V*km | km*64]
N_WO = 128 * D                 # 131072   [128, 1024]  Wo^T rows
N_RS = HD * QBLK               # 32768    [64, 512]    row scale (0/1)
N_BO = 128 * (D // 128)        # 1024     [128, 8]     bias
OFF = np.cumsum([0, N_QT, N_KN, N_VA, N_WO, N_RS, N_BO])
N_TOT = int(OFF[-1])


def _chunks(n_sk_tiles, parity):
    """Split sk tiles into PSUM-bank-sized chunks with an (size, tag) plan
    whose psum-slot reuse distance is always >=2, including across head
    boundaries: even heads run A,B,A,B,A and odd heads B,A,B,A,B."""
    if n_sk_tiles == 16:
        if parity == 0:
            return [(3, "A"), (4, "B"), (3, "A"), (4, "B"), (2, "A")]
        return [(4, "B"), (3, "A"), (4, "B"), (3, "A"), (2, "B")]
    out = []
    rem = n_sk_tiles
    tag = "A" if parity == 0 else "B"
    while rem > 0:
        c = min(3 if tag == "A" else 4, rem)
        out.append((c, tag))
        rem -= c
        tag = "B" if tag == "A" else "A"
    return out


def build_program():
    h, qblk, sk, d, hd = H, QBLK, SK, D, HD
    skt = sk // 128  # sk tiles (16)
    dch = d // 128  # d chunks (8)
    nj = d // 128  # output feature chunks (8)
    nc = bacc.Bacc(
        "TRN2",
        target_bir_lowering=False,
        debug=False,
        enable_asserts=False,
        num_devices=NCORES,
    )

    blob = nc.dram_tensor("blob", [N_TOT], FP16, kind="ExternalInput").ap()
    yt = nc.dram_tensor("yt", [qblk, d], FP16, kind="ExternalOutput").ap()

    qt = blob[int(OFF[0]) : int(OFF[1])].rearrange("(p x) -> p x", p=hd)
    kn = blob[int(OFF[1]) : int(OFF[2])].rearrange("(b p x) -> p b x", b=4, p=128)
    va_in = blob[int(OFF[2]) : int(OFF[3])]
    wo_in = blob[int(OFF[3]) : int(OFF[4])]
    rs = blob[int(OFF[4]) : int(OFF[5])].rearrange("(p x) -> p x", p=hd)
    bo = blob[int(OFF[5]) : int(OFF[6])].rearrange("(p x) -> p x", p=128)

    g4 = [[0, 1, 2, 3], [4, 5, 6, 7]]
    g8 = [[0, 1, 2, 3, 4, 5, 6, 7]]

    with tile.TileContext(nc) as tc:
        with (
            tc.tile_pool(name="const", bufs=1) as cpool,
            tc.tile_pool(name="stream", bufs=3) as spool,
            tc.tile_pool(name="exp", bufs=4) as epool,
            tc.tile_pool(name="drain", bufs=2) as dpool,
            tc.tile_pool(name="dram", bufs=1, space="DRAM") as gpool,
            tc.tile_pool(name="p3", bufs=1, space="PSUM") as p3,
            tc.tile_pool(name="p4", bufs=1, space="PSUM") as p4,
            tc.tile_pool(name="pacc", bufs=1, space="PSUM") as pacc,
        ):
            # ---- phase A: local prep + on-device redistribution ----
            identity = cpool.tile([128, 128], FP16)
            make_identity(nc, identity[:])

            kn_sb = cpool.tile([128, 4, d], FP16)
            nc.sync.dma_start(kn_sb[:], kn)

            ktT_sb = cpool.tile([128, dch, SLC], FP16)
            for half in range(2):
                tp = p4.tile([128, 2048], FP16, tag="qkB", name=f"tpK{half}")
                for k in range(16):
                    b = half * 2 + k // 8
                    dj = k % 8
                    nc.tensor.transpose(
                        tp[:, k * 128 : (k + 1) * 128],
                        kn_sb[:, b, dj * 128 : (dj + 1) * 128],
                        identity[:],
                    )
                    nc.vector.tensor_copy(
                        ktT_sb[:, dj, b * 128 : (b + 1) * 128],
                        tp[:, k * 128 : (k + 1) * 128],
                    )

            ktT_loc = gpool.tile([128, dch, SLC], FP16)
            nc.sync.dma_start(ktT_loc[:], ktT_sb[:])
            ktT_full = gpool.tile([4, 128, dch, SLC], FP16)
            nc.gpsimd.collective_compute(
                "AllGather",
                mybir.AluOpType.bypass,
                replica_groups=g4,
                ins=[ktT_loc[:]],
                outs=[ktT_full[:]],
            )

            va_loc = gpool.tile([SLC * (d + hd)], FP16)
            nc.sync.dma_start(va_loc[:], va_in)
            va_full_flat = gpool.tile([4 * SLC * (d + hd)], FP16)
            nc.gpsimd.collective_compute(
                "AllGather",
                mybir.AluOpType.bypass,
                replica_groups=g4,
                ins=[va_loc[:]],
                outs=[va_full_flat[:]],
            )
            va_full = va_full_flat[:].rearrange(
                "(r s m) -> r s m", r=4, s=SLC, m=d + hd
            )

            wo_loc = gpool.tile([128 * d], FP16)
            nc.sync.dma_start(wo_loc[:], wo_in)
            wo_full_flat = gpool.tile([dch * 128 * d], FP16, addr_space="Shared")
            nc.gpsimd.collective_compute(
                "AllGather",
                mybir.AluOpType.bypass,
                replica_groups=g8,
                ins=[wo_loc[:]],
                outs=[wo_full_flat[:]],
            )
            wo_full = wo_full_flat[:].rearrange("(c p j) -> p c j", c=dch, p=128)

            # ---- phase B: attention + output projection (per q block) ----
            qt_sb = cpool.tile([hd, h * qblk], FP16)
            nc.sync.dma_start(qt_sb[:], qt)
            outT_sb = cpool.tile([128, dch, qblk], FP16)
            wot_sb = cpool.tile([128, dch, d], FP16)
            nc.sync.dma_start(wot_sb[:], wo_full)
            bo_sb = cpool.tile([128, nj], FP16)
            nc.sync.dma_start(bo_sb[:], bo)
            rs_sb = cpool.tile([hd, qblk], FP16)
            nc.sync.dma_start(rs_sb[:], rs)

            kt_sbs, va_sbs, av_pss = {}, {}, {}

            def load_head(hh):
                kt_sbs[hh] = spool.tile([hd, skt, 128], FP16, tag="kt", name=f"kt_sb{hh}")
                src = ktT_full[:, (hh % 2) * hd : (hh % 2 + 1) * hd, hh // 2, :]
                nc.sync.dma_start(
                    kt_sbs[hh][:].rearrange("p (r t) x -> p r t x", r=4),
                    src.rearrange("r p (t x) -> p r t x", x=128),
                )
                va_sbs[hh] = spool.tile([128, skt, 128], FP16, tag="va", name=f"va_sb{hh}")
                vsrc = va_full[:, :, hh * hd : (hh + 1) * hd]
                nc.sync.dma_start(
                    va_sbs[hh][:, :, 0:hd].rearrange("p (r t) m -> p r t m", r=4),
                    vsrc.rearrange("r (t p) m -> p t r m", p=128).rearrange(
                        "p t r m -> p r t m"
                    ),
                )
                msrc = va_full[:, :, d : d + hd]
                nc.sync.dma_start(
                    va_sbs[hh][:, :, hd:128].rearrange("p (r t) m -> p r t m", r=4),
                    msrc.rearrange("r (t p) m -> p t r m", p=128).rearrange(
                        "p t r m -> p r t m"
                    ),
                )

            def drain_head(hh):
                # evacuate PSUM fast (frees the accumulation bank for the
                # next head), then softmax-denominator math from SBUF
                av_sb = dpool.tile([128, qblk], F32, tag="avsb")
                nc.vector.tensor_copy(av_sb[:], av_pss[hh][:])
                sc = dpool.tile([64, qblk], F32, tag="sc")
                nc.vector.tensor_scalar_add(sc[:], av_sb[64:128, :], 1e-30)
                nc.vector.reciprocal(sc[:], sc[:])
                nc.vector.tensor_mul(sc[:], sc[:], rs_sb[:])
                chunk, half = hh // 2, (hh % 2) * 64
                nc.vector.tensor_tensor(
                    outT_sb[half : half + 64, chunk, :],
                    av_sb[0:64, :],
                    sc[:],
                    mybir.AluOpType.mult,
                )

            # flat, software-pipelined chunk stream: QK(c+1) is emitted
            # before AV(c) so the in-order PE queue never waits on exp(c)
            chunks = []
            for hh in range(h):
                t0 = 0
                for csz, tag in _chunks(skt, hh % 2):
                    chunks.append((hh, t0, csz, tag))
                    t0 += csz

            load_head(0)
            load_head(1)

            def emit_av(item):
                ph, pt0, pcsz, pex = item
                for j in range(pcsz):
                    t = pt0 + j
                    nc.tensor.matmul(
                        av_pss[ph][:, :],
                        lhsT=va_sbs[ph][:, t, :],
                        rhs=pex[:, j * qblk : (j + 1) * qblk],
                        start=(t == 0),
                        stop=(t == skt - 1),
                    )
                if pt0 + pcsz == skt:
                    drain_head(ph)

            pending = []  # depth-2 queue of (hh, t0, csz, ex) awaiting AV
            for ci, (hh, t0, csz, tag) in enumerate(chunks):
                if t0 == 0:
                    if hh + 2 < h:
                        load_head(hh + 2)
                    av_pss[hh] = pacc.tile([128, qblk], F32, tag="acc", name=f"av_ps{hh}")
                pool = p3 if tag == "A" else p4
                qk_ps = pool.tile(
                    [128, csz * qblk], F32, tag="qk" + tag, name=f"qk_ps{ci}"
                )
                for j in range(csz):
                    t = t0 + j
                    nc.tensor.matmul(
                        qk_ps[:, j * qblk : (j + 1) * qblk],
                        lhsT=kt_sbs[hh][:, t, :],
                        rhs=qt_sb[:, hh * qblk : (hh + 1) * qblk],
                        start=True,
                        stop=True,
                    )
                if len(pending) == 2:
                    emit_av(pending.pop(0))
                ex = epool.tile([128, csz * qblk], FP16, tag="exp")
                nc.scalar.activation(ex[:], qk_ps[:], mybir.ActivationFunctionType.Exp)
                pending.append((hh, t0, csz, ex))

            for item in pending:
                emit_av(item)

            # output projection: yT[j,q] = sum_d WoT[d,j] outT[d,q] + bo[j],
            # then PE-transpose back to natural [q, j] layout for the output
            for jc in range(nj):
                if jc % 2 == 0:
                    y_ps = pacc.tile([128, qblk], F32, tag="acc")
                else:
                    y_ps = p3.tile([128, qblk], F32, tag="qkA")
                for dc in range(dch):
                    nc.tensor.matmul(
                        y_ps[:],
                        lhsT=wot_sb[:, dc, jc * 128 : (jc + 1) * 128],
                        rhs=outT_sb[:, dc, :],
                        start=(dc == 0),
                        stop=(dc == dch - 1),
                    )
                y_sb = dpool.tile([128, qblk], FP16, tag="y")
                nc.vector.tensor_tensor(
                    y_sb[:],
                    y_ps[:],
                    bo_sb[:, jc : jc + 1].to_broadcast((128, qblk)),
                    mybir.AluOpType.add,
                )
                ynat_sb = dpool.tile([128, 4, 128], FP16, tag="ynat")
                ytp = p4.tile([128, 512], FP16, tag="qkB", name=f"ytp{jc}")
                for qb in range(4):
                    nc.tensor.transpose(
                        ytp[:, qb * 128 : (qb + 1) * 128],
                        y_sb[:, qb * 128 : (qb + 1) * 128],
                        identity[:],
                    )
                    nc.vector.tensor_copy(
                        ynat_sb[:, qb, :], ytp[:, qb * 128 : (qb + 1) * 128]
                    )
                nc.sync.dma_start(
                    yt[:, jc * 128 : (jc + 1) * 128].rearrange(
                        "(b p) j -> p b j", p=128
                    ),
                    ynat_sb[:],
                )

    nc.compile()
    return nc


def shard_inputs(query, key, value, key_mask, query_mask, Wo, bo):
    """Full inputs -> per-core single-blob input maps (slices + casts only)."""
    km01 = (key_mask[:, :, 0] != 0)  # [B, SK] bool
    qm01 = (query_mask[:, :, 0] != 0)  # [B, SQ] bool
    any_km = km01.any(axis=1)  # [B]
    woT16 = np.ascontiguousarray(Wo.astype(np.float32).T).astype(np.float16)
    bo16 = np.ascontiguousarray(
        bo.astype(np.float32).reshape(D // 128, 128).T
    ).astype(np.float16)

    in_maps = []
    for core in range(NCORES):
        g, r = core // 4, core % 4
        rows = slice(r * QBLK, (r + 1) * QBLK)

        q_blk = query[g, rows, :].astype(np.float32) * 0.125
        qt = (
            q_blk.reshape(QBLK, H, HD).transpose(2, 1, 0).reshape(HD, H * QBLK)
        ).astype(np.float16)

        kn = key[g, rows, :].astype(np.float16)

        kmr = km01[g, rows]  # [512] bool
        vm = (value[g, rows, :] * kmr[:, None]).astype(np.float16)
        kmb = np.broadcast_to(
            kmr[:, None].astype(np.float16), (SLC, HD)
        )
        va = np.concatenate([vm, kmb], axis=1)  # [512, 1088]

        wos = woT16[core * 128 : (core + 1) * 128]  # [128, 1024]

        rsv = (qm01[g, rows] & bool(any_km[g])).astype(np.float16)  # [512]
        rsb = np.broadcast_to(rsv[None, :], (HD, QBLK))

        blob = np.concatenate(
            [
                qt.ravel(),
                kn.ravel(),
                va.ravel(),
                wos.ravel(),
                np.ascontiguousarray(rsb).ravel(),
                bo16.ravel(),
            ]
        )
        assert blob.size == N_TOT
        in_maps.append({"blob": blob})
    return in_maps


_NC_CACHE = {}


def _get_program():
    if "nc" not in _NC_CACHE:
        _NC_CACHE["nc"] = build_program()
    return _NC_CACHE["nc"]


def kernel(query, key, value, key_mask, query_mask, Wo, bo, _trace=False):
    query = np.asarray(query, dtype=np.float32)
    key = np.asarray(key, dtype=np.float32)
    value = np.asarray(value, dtype=np.float32)
    key_mask = np.asarray(key_mask, dtype=np.int32)
    query_mask = np.asarray(query_mask, dtype=np.int32)
    Wo = np.asarray(Wo, dtype=np.float32)
    bo = np.asarray(bo, dtype=np.float32)

    nc = _get_program()
    in_maps = shard_inputs(query, key, value, key_mask, query_mask, Wo, bo)
    try:
        res = bass_utils.run_bass_kernel_spmd(
            nc, in_maps, core_ids=list(range(NCORES)), trace=_trace
        )
    except ModuleNotFoundError:
        # axon NTFF profile hook unavailable in this container; run untraced
        res = bass_utils.run_bass_kernel_spmd(
            nc, in_maps, core_ids=list(range(NCORES)), trace=False
        )
    kernel.last_results = res

    out = np.empty((B, SQ, D), np.float32)
    for core in range(NCORES):
        g, r = core // 4, core % 4
        out[g, r * QBLK : (r + 1) * QBLK, :] = res.results[core]["yt"]
    return out
